# revision 39
# baseline (speedup 1.0000x reference)
"""CenterLossLayer Trainium2 kernel (8-core SPMD, Bass/Tile).

Strategy: shard by LABEL RANGE (12500 classes per core). The host sorts
samples by label (pure index manipulation) and packs them into per-band
chunks: band k covers classes [k*W, (k+1)*W); its samples occupy one
2048-slot chunk (16 tiles of 128), classes never straddling a tile. The
center table is stored band-blocked ([W class rows | D dump rows] per band)
so each chunk's scatter-add targets a STATIC DISJOINT slice of the output —
Tile's range-aware dependency tracking then runs all chunk scatters
independently (no WAW chain). All arithmetic happens on device:

  per tile of 128 sorted samples:
    E[j,k]   = (first_slot_of_group(j) == k)   (DVE is_equal vs const iota)
    d0       = centers[label] - features       (rows via dma_gather)
    loss_j   = sum_d d0^2                      (ACT Square + accum)
    S        = E^T @ [d0 || 1]                 (PE f32: group sums + counts
                                                at first-occurrence slots)
    value_k  = -ALPHA/(1+n_k) * S_k
  dma_scatter_add adds value rows into the band slice of new_centers
  (pre-initialized with centers). First-occurrence slots target their class
  row (globally unique -> no RMW races); other slots hit the band's dump
  rows. SWDGE ops round-robin the 4 descriptor queues so generation
  pipelines across Q7 core pairs.

kernel(**inputs) takes FULL inputs, returns (result[B,1], new_centers[C,D]).
"""

import sys

sys.path.insert(0, "/opt/trn_rl_repo")

import numpy as np

from concourse import bass, bacc, mybir
import concourse.tile as tile
from concourse.tile_rust import add_dep_helper
from concourse.bass_utils import run_bass_kernel_spmd

ALPHA = 0.5
NUM_CLASSES = 100000
FEAT_DIM = 128
BATCH = 131072
NCORES = 8

P = 128
TPC = 8           # tiles per chunk
CHUNK = TPC * P   # 1024 slots per chunk
DUMP = 128        # dump rows per band


class Cfg:
    def __init__(self, num_classes, batch, ncores, n_chunks):
        assert num_classes % ncores == 0
        self.C = num_classes
        self.B = batch
        self.ncores = ncores
        self.csh = num_classes // ncores   # classes per core
        self.n_chunks = n_chunks
        self.W = -(-self.csh // n_chunks)  # classes per band
        self.blk = self.W + DUMP           # table rows per band
        self.tbl = n_chunks * self.blk     # per-core table rows
        self.s_pad = n_chunks * CHUNK
        self.n_tiles = self.s_pad // P
        assert self.tbl < 32000, "int16 idx overflow"


# ----------------------------------------------------------------------------
# device program
# ----------------------------------------------------------------------------

def build_program(cfg: Cfg):
    nc = bacc.Bacc("TRN2", target_bir_lowering=False, debug=False,
                   num_devices=cfg.ncores, num_swdge_queues=4)
    f32 = mybir.dt.float32
    i16 = mybir.dt.int16
    T = cfg.n_tiles
    idx_cols = cfg.s_pad // 16

    feat = nc.declare_dram_parameter("feat", [P, T * P], f32, isOutput=False)
    ctr = nc.declare_dram_parameter("ctr", [cfg.tbl, P], f32, isOutput=False)
    gidx = nc.declare_dram_parameter("gidx", [P, idx_cols], i16, isOutput=False)
    sidx = nc.declare_dram_parameter("sidx", [P, idx_cols], i16, isOutput=False)
    foT = nc.declare_dram_parameter("foT", [P, T], f32, isOutput=False)
    iota_in = nc.declare_dram_parameter("iota16", [P, TPC * P], f32, isOutput=False)
    nctr = nc.declare_dram_parameter("nctr", [cfg.tbl, P], f32, isOutput=True)
    lout = nc.declare_dram_parameter("lout", [P, T], f32, isOutput=True)

    # SWDGE queue q runs on Q7 core pair q, so descriptor generation for ops
    # on different queues pipelines across pairs. Tile's DMASW sem lanes
    # (8, rotating in SCHEDULED order over Pool DMA ops) each lock to one
    # queue, so we pin the Pool-DMA dispatch order with no-sync edges and
    # assign queue = chain_position % 4 (lane i%8 <-> queue i%4). The chain
    # runs gathers LEAD chunks ahead of scatters so a scatter's sem wait
    # (its chunk's compute) never head-blocks upcoming gathers.
    LEAD = 5
    chain = [("dummy",)]  # static dispatch order of pool-DMA ops
    for k in range(cfg.n_chunks + LEAD):
        if k < cfg.n_chunks:
            chain.append(("g", k))
        if k >= LEAD:
            chain.append(("s", k - LEAD))
    # queue = f(position % 8). Steady state alternates gather (even pos) /
    # scatter (odd pos); scatter desc-gen costs ~2x gather, so this map gives
    # every queue one gather + one scatter per 4 chunks (balanced), while
    # staying a pure function of position so Tile's 8 rotating DMASW sem
    # lanes each see a single queue.
    F = [0, 1, 2, 3, 1, 0, 3, 2]
    qmap = {op: F[i % 8] for i, op in enumerate(chain)}
    emitted = {}

    with tile.TileContext(nc) as tc:
        with (
            tc.tile_pool(name="const", bufs=1) as cpool,
            tc.tile_pool(name="io", bufs=7) as iopool,
            tc.tile_pool(name="work", bufs=3) as wpool,
            tc.tile_pool(name="sct", bufs=3) as spool,
            tc.tile_pool(name="ps_m", bufs=4, space="PSUM") as psm,
        ):
            # small constant loads first (sync HWDGE ring), then per-band
            # init copies, then the feature chunk streams. gidx is split so
            # the first gather only waits on its own tiny slice.
            gix = cpool.tile([P, idx_cols], i16)
            c0 = CHUNK // 16
            nc.sync.dma_start(out=gix[:, :c0], in_=gidx[:, :c0])
            nc.sync.dma_start(out=gix[:, c0:], in_=gidx[:, c0:])
            iota = cpool.tile([P, TPC * P], f32)
            nc.sync.dma_start(out=iota[:], in_=iota_in[:])
            fo = cpool.tile([P, T], f32)
            nc.sync.dma_start(out=fo[:], in_=foT[:])
            six = cpool.tile([P, idx_cols], i16)
            nc.sync.dma_start(out=six[:], in_=sidx[:])
            losbuf = cpool.tile([P, T], f32)

            # tiny dummy gather issued first: triggers the ~6us mlp-library
            # IRAM load while the real index tensors are still streaming in
            dzi = cpool.tile([P, 8], i16)
            nc.vector.memset(dzi[:], 0)
            dzo = cpool.tile([P, 1, P], f32)
            emitted[("dummy",)] = nc.gpsimd.dma_gather(
                out_ap=dzo[:], in_ap=ctr[:], idxs_ap=dzi[:],
                num_idxs=P, num_idxs_reg=P, elem_size=P,
                queue_num=qmap[("dummy",)],
            )

            # new_centers := centers, per band, so each chunk's scatter only
            # waits on its own band's init; on the scalar HWDGE ring so the
            # sync ring stays free for the feature stream
            for k in range(cfg.n_chunks):
                sl = slice(k * cfg.blk, (k + 1) * cfg.blk)
                nc.scalar.dma_start(out=nctr[sl], in_=ctr[sl])

            for kc in range(cfg.n_chunks):
                sct = spool.tile([P, TPC, P], f32, tag="sct")
                fk = iopool.tile([P, TPC, P], f32, tag="fk")
                nc.sync.dma_start(
                    out=fk[:], in_=feat[:, kc * CHUNK : (kc + 1) * CHUNK]
                )
                ck = iopool.tile([P, TPC, P], f32, tag="ck")
                emitted[("g", kc)] = nc.gpsimd.dma_gather(
                    out_ap=ck[:],
                    in_ap=ctr[:],
                    idxs_ap=gix[:, kc * TPC * 8 : (kc + 1) * TPC * 8],
                    num_idxs=CHUNK,
                    num_idxs_reg=CHUNK,
                    elem_size=P,
                    queue_num=qmap[("g", kc)],
                )
                # E[j, t, k] = (first_slot(j, tile t) == k), whole chunk at once
                E = wpool.tile([P, TPC, P], f32, tag="E")
                nc.vector.tensor_tensor(
                    out=E[:],
                    in0=fo[:, kc * TPC : (kc + 1) * TPC].to_broadcast([P, TPC, P]),
                    in1=iota[:].rearrange("p (t q) -> p t q", t=TPC),
                    op=mybir.AluOpType.is_equal,
                )
                # d0e = [centers_row - feature || 1], whole chunk at once
                d0e = wpool.tile([P, TPC, P + 1], f32, tag="d0e")
                nc.vector.memset(d0e[:, :, P : P + 1], 1.0)
                nc.vector.tensor_tensor(
                    out=d0e[:, :, :P],
                    in0=ck[:],
                    in1=fk[:],
                    op=mybir.AluOpType.subtract,
                )
                for tj in range(TPC // 2):  # pairs of tiles share a PSUM bank
                    S2 = psm.tile([P, 2, P + 1], f32, space="PSUM", tag="S2")
                    for i in range(2):
                        t = 2 * tj + i
                        gt = kc * TPC + t
                        nc.tensor.matmul(
                            out=S2[:, i, :], lhsT=E[:, t, :], rhs=d0e[:, t, :],
                            start=True, stop=True,
                        )
                        # loss = sum(d0^2) along free dim (ACT square+accum)
                        sq = wpool.tile([P, P], f32, tag="sq")
                        nc.scalar.activation(
                            out=sq[:],
                            in_=d0e[:, t, :P],
                            func=mybir.ActivationFunctionType.Square,
                            accum_out=losbuf[:, gt : gt + 1],
                        )
                    # r = 1/(1+n); scatter values = -ALPHA * S * r
                    n1 = wpool.tile([P, 2], f32, tag="n1")
                    nc.vector.tensor_scalar(
                        out=n1[:], in0=S2[:, :, P : P + 1], scalar1=1.0,
                        scalar2=None, op0=mybir.AluOpType.add,
                    )
                    rv = wpool.tile([P, 2], f32, tag="rv")
                    nc.vector.reciprocal(out=rv[:], in_=n1[:])
                    nc.vector.scalar_tensor_tensor(
                        out=sct[:, 2 * tj : 2 * tj + 2, :],
                        in0=S2[:, :, :P],
                        scalar=-ALPHA,
                        in1=rv[:, :, None].to_broadcast([P, 2, P]),
                        op0=mybir.AluOpType.mult,
                        op1=mybir.AluOpType.mult,
                    )
                emitted[("s", kc)] = nc.gpsimd.dma_scatter_add(
                    out_ap=nctr[kc * cfg.blk : (kc + 1) * cfg.blk],
                    in_ap=sct[:],
                    idxs_ap=six[:, kc * TPC * 8 : (kc + 1) * TPC * 8],
                    num_idxs=CHUNK,
                    num_idxs_reg=CHUNK,
                    elem_size=P,
                    queue_num=qmap[("s", kc)],
                )
            for a, b in zip(chain[1:], chain[:-1]):
                add_dep_helper(emitted[a].ins, emitted[b].ins, sync=False,
                               reason="pin pool-dma order for queue/lane pairing")
            nc.scalar.dma_start(out=lout[:], in_=losbuf[:])
    nc.finalize()
    return nc


# ----------------------------------------------------------------------------
# host sharding / packing
# ----------------------------------------------------------------------------

def host_pack(labels: np.ndarray, ncores: int, csh: int):
    """Sort by label, range-shard, band-align: band k (classes [kW,(k+1)W))
    fills chunk k (2048 slots, 16 straddle-free tiles). Pure index work.

    Returns (cores metadata, n_chunks)."""
    labels = np.asarray(labels).reshape(-1).astype(np.int64)
    order = np.argsort(labels, kind="stable")
    slab = labels[order]
    bounds = np.searchsorted(slab, np.arange(ncores + 1) * csh)

    pre = []
    for c in range(ncores):
        lo, hi = bounds[c], bounds[c + 1]
        samp = order[lo:hi]
        lab = slab[lo:hi] - c * csh
        starts = np.flatnonzero(np.r_[True, lab[1:] != lab[:-1]])
        lens = np.diff(np.r_[starts, len(lab)])
        assert lens.max(initial=0) <= P
        pre.append((samp, lab, starts, lens))

    def try_pack(n_chunks):
        W = -(-csh // n_chunks)
        out = []
        for samp, lab, starts, lens in pre:
            run_lab = lab[starts]
            slot = np.empty(len(lab), np.int64)
            for k in range(n_chunks):
                rsel = np.flatnonzero((run_lab >= k * W) & (run_lab < (k + 1) * W))
                cur = k * CHUNK
                limit = (k + 1) * CHUNK
                for ri in rsel.tolist():
                    s, L = starts[ri], lens[ri]
                    room = P - (cur % P)
                    if L > room:
                        cur += room
                    if cur + L > limit:
                        return None
                    slot[s : s + L] = np.arange(cur, cur + L)
                    cur += L
            out.append(slot)
        return out

    n_chunks = max(1, -(-len(labels) // (ncores * CHUNK)))
    while True:
        slots = try_pack(n_chunks)
        if slots is not None:
            break
        n_chunks += 1
        assert n_chunks <= 64

    s_pad = n_chunks * CHUNK
    cores = []
    for c in range(ncores):
        samp, lab, starts, lens = pre[c]
        slot = slots[c]
        samp_at = np.full(s_pad, -1, np.int64)
        samp_at[slot] = samp
        real = samp_at >= 0

        gidx = np.zeros(s_pad, np.int16)  # padded-table row per slot
        lab_at = np.zeros(s_pad, np.int64)
        lab_at[slot] = lab
        W = -(-csh // n_chunks)
        pad_row = lab_at // W * (W + DUMP) + lab_at % W
        gidx[real] = pad_row[real].astype(np.int16)

        sl = np.arange(s_pad)
        fo = (sl % P).astype(np.int64)
        first_of = slot[starts].repeat(lens)
        fo[slot] = first_of % P

        sct = (W + (sl % DUMP)).astype(np.int16)  # band-relative dump rows
        sct[slot[starts]] = (lab[starts] % W).astype(np.int16)

        cores.append(
            dict(samp_at=samp_at, real=real, gidx=gidx,
                 fo=fo.astype(np.float32), sct=sct)
        )
    return cores, n_chunks


def _wrap_idx(a: np.ndarray) -> np.ndarray:
    """[S] int16 -> [128, S/16] wrapped layout replicated to 8 groups."""
    w = a.reshape(-1, 16).T
    return np.tile(w, (8, 1)).copy()


def make_in_maps(features, centers, cores, cfg: Cfg):
    features = np.asarray(features, dtype=np.float32)
    centers = np.asarray(centers, dtype=np.float32)
    T = cfg.n_tiles
    in_maps = []
    iota16 = np.tile(np.arange(P, dtype=np.float32), (P, TPC))
    for c, m in enumerate(cores):
        fs = np.zeros((cfg.s_pad, P), np.float32)
        fs[m["real"]] = features[m["samp_at"][m["real"]]]
        feat_sw = np.ascontiguousarray(
            fs.reshape(T, P, P).transpose(1, 0, 2).reshape(P, T * P)
        )
        ctab = np.zeros((cfg.tbl, P), np.float32)
        base = c * cfg.csh
        for k in range(cfg.n_chunks):
            w = min(cfg.W, cfg.csh - k * cfg.W)
            ctab[k * cfg.blk : k * cfg.blk + w] = \
                centers[base + k * cfg.W : base + k * cfg.W + w]
        in_maps.append(
            {
                "feat": feat_sw,
                "ctr": ctab,
                "gidx": _wrap_idx(m["gidx"]),
                "sidx": _wrap_idx(m["sct"]),
                "foT": np.ascontiguousarray(m["fo"].reshape(T, P).T),
                "iota16": iota16,
            }
        )
    return in_maps


def unshard(results, cores, cfg: Cfg):
    result = np.empty((cfg.B, 1), np.float32)
    new_centers = np.empty((cfg.C, P), np.float32)
    for c, (res, m) in enumerate(zip(results, cores)):
        nt = res["nctr"]
        base = c * cfg.csh
        for k in range(cfg.n_chunks):
            w = min(cfg.W, cfg.csh - k * cfg.W)
            new_centers[base + k * cfg.W : base + k * cfg.W + w] = \
                nt[k * cfg.blk : k * cfg.blk + w]
        loss_sorted = res["lout"].T.reshape(cfg.s_pad)
        real = m["real"]
        result[m["samp_at"][real], 0] = loss_sorted[real]
    return result, new_centers


# ----------------------------------------------------------------------------
# entry point
# ----------------------------------------------------------------------------

_NC_CACHE = {}


def _get_nc(cfg: Cfg):
    key = (cfg.C, cfg.B, cfg.n_chunks)
    if key not in _NC_CACHE:
        _NC_CACHE[key] = build_program(cfg)
    return _NC_CACHE[key]


def run(features, labels, centers, num_classes=NUM_CLASSES, **spmd_kwargs):
    cores, n_chunks = host_pack(labels, NCORES, num_classes // NCORES)
    cfg = Cfg(num_classes, len(np.asarray(labels).reshape(-1)), NCORES, n_chunks)
    in_maps = make_in_maps(features, centers, cores, cfg)
    nc = _get_nc(cfg)
    br = run_bass_kernel_spmd(nc, in_maps, list(range(cfg.ncores)), **spmd_kwargs)
    result, new_centers = unshard(br.results, cores, cfg)
    return result, new_centers, br


def kernel(features, labels, centers):
    result, new_centers, _ = run(features, labels, centers)
    return result, new_centers


# revision 40
# speedup vs baseline: 1.1491x; 1.1491x over previous
"""CenterLossLayer Trainium2 kernel (8-core SPMD, Bass/Tile).

Strategy: shard by LABEL RANGE (12500 classes per core). The host sorts
samples by label (pure index manipulation) and packs them into per-band
chunks: band k covers classes [k*W, (k+1)*W); its samples occupy one
2048-slot chunk (16 tiles of 128), classes never straddling a tile. The
center table is stored band-blocked ([W class rows | D dump rows] per band)
so each chunk's scatter-add targets a STATIC DISJOINT slice of the output —
Tile's range-aware dependency tracking then runs all chunk scatters
independently (no WAW chain). All arithmetic happens on device:

  per tile of 128 sorted samples:
    E[j,k]   = (first_slot_of_group(j) == k)   (DVE is_equal vs const iota)
    d0       = centers[label] - features       (rows via dma_gather)
    loss_j   = sum_d d0^2                      (ACT Square + accum)
    S        = E^T @ [d0 || 1]                 (PE f32: group sums + counts
                                                at first-occurrence slots)
    value_k  = -ALPHA/(1+n_k) * S_k
  dma_scatter_add adds value rows into the band slice of new_centers
  (pre-initialized with centers). First-occurrence slots target their class
  row (globally unique -> no RMW races); other slots hit the band's dump
  rows. SWDGE ops round-robin the 4 descriptor queues so generation
  pipelines across Q7 core pairs.

kernel(**inputs) takes FULL inputs, returns (result[B,1], new_centers[C,D]).
"""

import sys

sys.path.insert(0, "/opt/trn_rl_repo")

import numpy as np

from concourse import bass, bacc, mybir
import concourse.tile as tile
from concourse.tile_rust import add_dep_helper
from concourse.bass_utils import run_bass_kernel_spmd

ALPHA = 0.5
NUM_CLASSES = 100000
FEAT_DIM = 128
BATCH = 131072
NCORES = 8

P = 128
TPC = 8           # tiles per chunk
CHUNK = TPC * P   # 1024 slots per chunk
DUMP = 128        # dump rows per band


class Cfg:
    def __init__(self, num_classes, batch, ncores, n_chunks):
        assert num_classes % ncores == 0
        self.C = num_classes
        self.B = batch
        self.ncores = ncores
        self.csh = num_classes // ncores   # classes per core
        self.n_chunks = n_chunks
        self.W = -(-self.csh // n_chunks)  # classes per band
        self.blk = self.W + DUMP           # table rows per band
        self.tbl = n_chunks * self.blk     # per-core table rows
        self.s_pad = n_chunks * CHUNK
        self.n_tiles = self.s_pad // P
        assert self.tbl < 32000, "int16 idx overflow"


# ----------------------------------------------------------------------------
# device program
# ----------------------------------------------------------------------------

def build_program(cfg: Cfg):
    nc = bacc.Bacc("TRN2", target_bir_lowering=False, debug=False,
                   num_devices=cfg.ncores, num_swdge_queues=4)
    f32 = mybir.dt.float32
    i16 = mybir.dt.int16
    T = cfg.n_tiles
    idx_cols = cfg.s_pad // 16

    feat = nc.declare_dram_parameter("feat", [P, T * P], f32, isOutput=False)
    ctr = nc.declare_dram_parameter("ctr", [cfg.tbl, P], f32, isOutput=False)
    gidx = nc.declare_dram_parameter("gidx", [P, idx_cols], i16, isOutput=False)
    sidx = nc.declare_dram_parameter("sidx", [P, idx_cols], i16, isOutput=False)
    foT = nc.declare_dram_parameter("foT", [P, T], f32, isOutput=False)
    iota_in = nc.declare_dram_parameter("iota16", [P, TPC * P], f32, isOutput=False)
    nctr = nc.declare_dram_parameter("nctr", [cfg.tbl, P], f32, isOutput=True)
    lout = nc.declare_dram_parameter("lout", [P, T], f32, isOutput=True)

    # SWDGE queue q runs on Q7 core pair q, so descriptor generation for ops
    # on different queues pipelines across pairs. Tile's DMASW sem lanes
    # (8, rotating in SCHEDULED order over Pool DMA ops) each lock to one
    # queue, so we pin the Pool-DMA dispatch order with no-sync edges and
    # assign queue = chain_position % 4 (lane i%8 <-> queue i%4). The chain
    # runs gathers LEAD chunks ahead of scatters so a scatter's sem wait
    # (its chunk's compute) never head-blocks upcoming gathers.
    LEAD = 4
    chain = [("dummy",)]  # static dispatch order of pool-DMA ops
    for k in range(cfg.n_chunks + LEAD):
        if k < cfg.n_chunks:
            chain.append(("g", k))
        if k >= LEAD:
            chain.append(("s", k - LEAD))
    # queue = f(position % 8). Steady state alternates gather (even pos) /
    # scatter (odd pos); scatter desc-gen costs ~2x gather, so this map gives
    # every queue one gather + one scatter per 4 chunks (balanced), while
    # staying a pure function of position so Tile's 8 rotating DMASW sem
    # lanes each see a single queue.
    F = [0, 1, 2, 3, 1, 0, 3, 2]
    qmap = {op: F[i % 8] for i, op in enumerate(chain)}
    emitted = {}

    with tile.TileContext(nc) as tc:
        with (
            tc.tile_pool(name="const", bufs=1) as cpool,
            tc.tile_pool(name="io", bufs=6) as iopool,
            tc.tile_pool(name="work", bufs=3) as wpool,
            tc.tile_pool(name="sct", bufs=6) as spool,
            tc.tile_pool(name="ps_m", bufs=4, space="PSUM") as psm,
        ):
            # small constant loads first (sync HWDGE ring), then per-band
            # init copies, then the feature chunk streams. gidx is split so
            # the first gather only waits on its own tiny slice.
            gix = cpool.tile([P, idx_cols], i16)
            c0 = CHUNK // 16
            nc.sync.dma_start(out=gix[:, :c0], in_=gidx[:, :c0])
            nc.sync.dma_start(out=gix[:, c0:], in_=gidx[:, c0:])
            iota = cpool.tile([P, TPC * P], f32)
            nc.sync.dma_start(out=iota[:], in_=iota_in[:])
            fo = cpool.tile([P, T], f32)
            nc.sync.dma_start(out=fo[:], in_=foT[:])
            six = cpool.tile([P, idx_cols], i16)
            nc.sync.dma_start(out=six[:], in_=sidx[:])
            losbuf = cpool.tile([P, T], f32)

            # tiny dummy gather issued first: triggers the ~6us mlp-library
            # IRAM load while the real index tensors are still streaming in
            dzi = cpool.tile([P, 8], i16)
            nc.vector.memset(dzi[:], 0)
            dzo = cpool.tile([P, 1, P], f32)
            emitted[("dummy",)] = nc.gpsimd.dma_gather(
                out_ap=dzo[:], in_ap=ctr[:], idxs_ap=dzi[:],
                num_idxs=P, num_idxs_reg=P, elem_size=P,
                queue_num=qmap[("dummy",)],
            )

            # new_centers := centers, per band, so each chunk's scatter only
            # waits on its own band's init; on the scalar HWDGE ring so the
            # sync ring stays free for the feature stream
            for k in range(cfg.n_chunks):
                sl = slice(k * cfg.blk, (k + 1) * cfg.blk)
                nc.scalar.dma_start(out=nctr[sl], in_=ctr[sl])

            for kc in range(cfg.n_chunks):
                sct = spool.tile([P, TPC, P], f32, tag="sct")
                fk = iopool.tile([P, TPC, P], f32, tag="fk")
                nc.sync.dma_start(
                    out=fk[:], in_=feat[:, kc * CHUNK : (kc + 1) * CHUNK]
                )
                ck = iopool.tile([P, TPC, P], f32, tag="ck")
                emitted[("g", kc)] = nc.gpsimd.dma_gather(
                    out_ap=ck[:],
                    in_ap=ctr[:],
                    idxs_ap=gix[:, kc * TPC * 8 : (kc + 1) * TPC * 8],
                    num_idxs=CHUNK,
                    num_idxs_reg=CHUNK,
                    elem_size=P,
                    queue_num=qmap[("g", kc)],
                )
                # E[j, t, k] = (first_slot(j, tile t) == k), whole chunk at once
                E = wpool.tile([P, TPC, P], f32, tag="E")
                nc.vector.tensor_tensor(
                    out=E[:],
                    in0=fo[:, kc * TPC : (kc + 1) * TPC].to_broadcast([P, TPC, P]),
                    in1=iota[:].rearrange("p (t q) -> p t q", t=TPC),
                    op=mybir.AluOpType.is_equal,
                )
                # d0e = [centers_row - feature || 1], whole chunk at once
                d0e = wpool.tile([P, TPC, P + 1], f32, tag="d0e")
                nc.vector.memset(d0e[:, :, P : P + 1], 1.0)
                nc.vector.tensor_tensor(
                    out=d0e[:, :, :P],
                    in0=ck[:],
                    in1=fk[:],
                    op=mybir.AluOpType.subtract,
                )
                for tj in range(TPC // 2):  # pairs of tiles share a PSUM bank
                    S2 = psm.tile([P, 2, P + 1], f32, space="PSUM", tag="S2")
                    for i in range(2):
                        t = 2 * tj + i
                        gt = kc * TPC + t
                        nc.tensor.matmul(
                            out=S2[:, i, :], lhsT=E[:, t, :], rhs=d0e[:, t, :],
                            start=True, stop=True,
                        )
                        # loss = sum(d0^2) along free dim (ACT square+accum)
                        sq = wpool.tile([P, P], f32, tag="sq")
                        nc.scalar.activation(
                            out=sq[:],
                            in_=d0e[:, t, :P],
                            func=mybir.ActivationFunctionType.Square,
                            accum_out=losbuf[:, gt : gt + 1],
                        )
                    # r = 1/(1+n); scatter values = -ALPHA * S * r
                    n1 = wpool.tile([P, 2], f32, tag="n1")
                    nc.vector.tensor_scalar(
                        out=n1[:], in0=S2[:, :, P : P + 1], scalar1=1.0,
                        scalar2=None, op0=mybir.AluOpType.add,
                    )
                    rv = wpool.tile([P, 2], f32, tag="rv")
                    nc.vector.reciprocal(out=rv[:], in_=n1[:])
                    nc.vector.scalar_tensor_tensor(
                        out=sct[:, 2 * tj : 2 * tj + 2, :],
                        in0=S2[:, :, :P],
                        scalar=-ALPHA,
                        in1=rv[:, :, None].to_broadcast([P, 2, P]),
                        op0=mybir.AluOpType.mult,
                        op1=mybir.AluOpType.mult,
                    )
                emitted[("s", kc)] = nc.gpsimd.dma_scatter_add(
                    out_ap=nctr[kc * cfg.blk : (kc + 1) * cfg.blk],
                    in_ap=sct[:],
                    idxs_ap=six[:, kc * TPC * 8 : (kc + 1) * TPC * 8],
                    num_idxs=CHUNK,
                    num_idxs_reg=CHUNK,
                    elem_size=P,
                    queue_num=qmap[("s", kc)],
                )
            for a, b in zip(chain[1:], chain[:-1]):
                add_dep_helper(emitted[a].ins, emitted[b].ins, sync=False,
                               reason="pin pool-dma order for queue/lane pairing")
            nc.scalar.dma_start(out=lout[:], in_=losbuf[:])
    nc.finalize()
    return nc


# ----------------------------------------------------------------------------
# host sharding / packing
# ----------------------------------------------------------------------------

def host_pack(labels: np.ndarray, ncores: int, csh: int):
    """Sort by label, range-shard, band-align: band k (classes [kW,(k+1)W))
    fills chunk k (2048 slots, 16 straddle-free tiles). Pure index work.

    Returns (cores metadata, n_chunks)."""
    labels = np.asarray(labels).reshape(-1).astype(np.int64)
    order = np.argsort(labels, kind="stable")
    slab = labels[order]
    bounds = np.searchsorted(slab, np.arange(ncores + 1) * csh)

    pre = []
    for c in range(ncores):
        lo, hi = bounds[c], bounds[c + 1]
        samp = order[lo:hi]
        lab = slab[lo:hi] - c * csh
        starts = np.flatnonzero(np.r_[True, lab[1:] != lab[:-1]])
        lens = np.diff(np.r_[starts, len(lab)])
        assert lens.max(initial=0) <= P
        pre.append((samp, lab, starts, lens))

    def try_pack(n_chunks):
        W = -(-csh // n_chunks)
        out = []
        for samp, lab, starts, lens in pre:
            run_lab = lab[starts]
            slot = np.empty(len(lab), np.int64)
            for k in range(n_chunks):
                rsel = np.flatnonzero((run_lab >= k * W) & (run_lab < (k + 1) * W))
                cur = k * CHUNK
                limit = (k + 1) * CHUNK
                for ri in rsel.tolist():
                    s, L = starts[ri], lens[ri]
                    room = P - (cur % P)
                    if L > room:
                        cur += room
                    if cur + L > limit:
                        return None
                    slot[s : s + L] = np.arange(cur, cur + L)
                    cur += L
            out.append(slot)
        return out

    n_chunks = max(1, -(-len(labels) // (ncores * CHUNK)))
    while True:
        slots = try_pack(n_chunks)
        if slots is not None:
            break
        n_chunks += 1
        assert n_chunks <= 64

    s_pad = n_chunks * CHUNK
    cores = []
    for c in range(ncores):
        samp, lab, starts, lens = pre[c]
        slot = slots[c]
        samp_at = np.full(s_pad, -1, np.int64)
        samp_at[slot] = samp
        real = samp_at >= 0

        gidx = np.zeros(s_pad, np.int16)  # padded-table row per slot
        lab_at = np.zeros(s_pad, np.int64)
        lab_at[slot] = lab
        W = -(-csh // n_chunks)
        pad_row = lab_at // W * (W + DUMP) + lab_at % W
        gidx[real] = pad_row[real].astype(np.int16)

        sl = np.arange(s_pad)
        fo = (sl % P).astype(np.int64)
        first_of = slot[starts].repeat(lens)
        fo[slot] = first_of % P

        sct = (W + (sl % DUMP)).astype(np.int16)  # band-relative dump rows
        sct[slot[starts]] = (lab[starts] % W).astype(np.int16)

        cores.append(
            dict(samp_at=samp_at, real=real, gidx=gidx,
                 fo=fo.astype(np.float32), sct=sct)
        )
    return cores, n_chunks


def _wrap_idx(a: np.ndarray) -> np.ndarray:
    """[S] int16 -> [128, S/16] wrapped layout replicated to 8 groups."""
    w = a.reshape(-1, 16).T
    return np.tile(w, (8, 1)).copy()


def make_in_maps(features, centers, cores, cfg: Cfg):
    features = np.asarray(features, dtype=np.float32)
    centers = np.asarray(centers, dtype=np.float32)
    T = cfg.n_tiles
    in_maps = []
    iota16 = np.tile(np.arange(P, dtype=np.float32), (P, TPC))
    for c, m in enumerate(cores):
        fs = np.zeros((cfg.s_pad, P), np.float32)
        fs[m["real"]] = features[m["samp_at"][m["real"]]]
        feat_sw = np.ascontiguousarray(
            fs.reshape(T, P, P).transpose(1, 0, 2).reshape(P, T * P)
        )
        ctab = np.zeros((cfg.tbl, P), np.float32)
        base = c * cfg.csh
        for k in range(cfg.n_chunks):
            w = min(cfg.W, cfg.csh - k * cfg.W)
            ctab[k * cfg.blk : k * cfg.blk + w] = \
                centers[base + k * cfg.W : base + k * cfg.W + w]
        in_maps.append(
            {
                "feat": feat_sw,
                "ctr": ctab,
                "gidx": _wrap_idx(m["gidx"]),
                "sidx": _wrap_idx(m["sct"]),
                "foT": np.ascontiguousarray(m["fo"].reshape(T, P).T),
                "iota16": iota16,
            }
        )
    return in_maps


def unshard(results, cores, cfg: Cfg):
    result = np.empty((cfg.B, 1), np.float32)
    new_centers = np.empty((cfg.C, P), np.float32)
    for c, (res, m) in enumerate(zip(results, cores)):
        nt = res["nctr"]
        base = c * cfg.csh
        for k in range(cfg.n_chunks):
            w = min(cfg.W, cfg.csh - k * cfg.W)
            new_centers[base + k * cfg.W : base + k * cfg.W + w] = \
                nt[k * cfg.blk : k * cfg.blk + w]
        loss_sorted = res["lout"].T.reshape(cfg.s_pad)
        real = m["real"]
        result[m["samp_at"][real], 0] = loss_sorted[real]
    return result, new_centers


# ----------------------------------------------------------------------------
# entry point
# ----------------------------------------------------------------------------

_NC_CACHE = {}


def _get_nc(cfg: Cfg):
    key = (cfg.C, cfg.B, cfg.n_chunks)
    if key not in _NC_CACHE:
        _NC_CACHE[key] = build_program(cfg)
    return _NC_CACHE[key]


def run(features, labels, centers, num_classes=NUM_CLASSES, **spmd_kwargs):
    cores, n_chunks = host_pack(labels, NCORES, num_classes // NCORES)
    cfg = Cfg(num_classes, len(np.asarray(labels).reshape(-1)), NCORES, n_chunks)
    in_maps = make_in_maps(features, centers, cores, cfg)
    nc = _get_nc(cfg)
    br = run_bass_kernel_spmd(nc, in_maps, list(range(cfg.ncores)), **spmd_kwargs)
    result, new_centers = unshard(br.results, cores, cfg)
    return result, new_centers, br


def kernel(features, labels, centers):
    result, new_centers, _ = run(features, labels, centers)
    return result, new_centers
